# revision 28
# baseline (speedup 1.0000x reference)
"""MoD router Trainium2 kernel (fp16-stream + exact-refinement version).

Computes, for hidden_states [4, 4096, 2048] and gate_w [1, 2048]:
    scores = einsum("bsh,h->bs", hidden_states, gate_w[0])        # [4, 4096]
    mask   = top-k mask per batch row (k = 2048 = S/2), 1.0/0.0   # [4, 4096]
returns (mask, scores), matching the reference.

The f32 kernel was pinned at the SBUF-fabric DMA ceiling (16.78 MB/core at
~436 GB/s = 38 us).  This version halves the stream: the host uploads the
per-core hidden slab as fp16 (8.39 MB), the DVE matvec runs on fp16
operands (f32 accumulate), and mask exactness vs the f32 reference is
restored with a refinement pass:

  1. Stream fp16 slab, DVE matvec -> approx scores s^ (err sigma ~7e-4);
     pairwise AllGather so each core holds its full 4096-score batch row.
  2. 2-level 127-ary search -> tau^ (within 1.3e-4 of the true median).
  3. Entries with |s^ - tau^| <= DELTA form the band (~40 of 4096; all
     possible misclassifications live there, DELTA >> max score error).
     Entries with s^ > tau^ + DELTA are certainly in the top-k.
  4. Band entry ids are compacted with gpsimd sparse_gather, their f32
     rows indirect-DMA-gathered from an f32 copy of the full batch row
     (uploaded host-side), rescored exactly on DVE.
  5. A 3-level 127-ary search over the <=128 exact band scores finds the
     (K - #above)-th largest (final cell 1.5e-8 < score spacing).  The
     level pivots live along the free axis as affine host constants and
     the cross-partition count is a ones-matmul on 0/1 indicators
     (integer-exact in fp32r); band bits return to their own-half flat
     positions via a one-hot is_equal matmul.  Everything stays in
     SBUF/PSUM: no partition_broadcast (second Q7 library -> reload race
     on HW) and no DRAM scatter round-trip.  Result: bit-exact top-k
     mask on the reference input.

Distribution: unchanged from the f32 version: B*S rows sharded 8 ways,
cores 2b/2b+1 own the halves of batch row b and exchange scores with a
pairwise AllGather (8 KB).  Both cores of a pair run the identical
deterministic search on identical data, so their decisions agree.

Timing probes: _MATVEC_COLS=1 strips DVE matvec work but keeps the DMA
stream; _REPS replicates the body inside one NEFF for delta timing.
"""

import numpy as np

B, S, H = 4, 4096, 2048
N_CORES = 8
R = (B * S) // N_CORES      # rows per core = 2048
RT = R // 128               # 128-row tiles per core = 16
K_TOP = S // 2              # 2048
NA = 2                      # row-tiles per DMA chunk (chunk = NA/2 MiB fp16)
N_LEVELS = 2                # tau^ search levels (127-ary, bracket [-1,1])
W0 = 2.0
CS = [W0 / (127.0 ** (l + 1)) for l in range(N_LEVELS)] + [0.0]
N_ACT = 3584                # cols counted on ACT in tau^ search (rest DVE)
COMBINE_K = float(2 * K_TOP - N_ACT)
DELTA = 0.012               # refinement band half-width (>> score err)
DPRIME = 1.25 * DELTA       # refinement search bracket half-width
N_REF = 3                   # refinement search levels
CSR = [2.0 * DPRIME / (127.0 ** (l + 1)) for l in range(N_REF)] + [0.0]
BIG = 1.0e30

_CACHE = {}
_DBG = False  # add debug DRAM outputs for refinement intermediates
_REPS = 1   # repeat whole body inside one NEFF (timing aid)
_MATVEC_COLS = H  # timing probe: set to 1 to keep DMA but skip DVE work
_H_RINGS = 2      # DMA rings for h chunks: sync+scalar HWDGE
_HP_BUFS = 0      # h-tile lookahead bufs (0 = default max(2, 10//NA))


def _build_nc():
    import concourse.bacc as bacc
    import concourse.tile as tile
    import concourse.mybir as mybir
    from concourse.bass import IndirectOffsetOnAxis

    f32 = mybir.dt.float32
    f16 = mybir.dt.float16
    i32 = mybir.dt.int32
    u32 = mybir.dt.uint32
    Alu = mybir.AluOpType
    Act = mybir.ActivationFunctionType

    nc = bacc.Bacc("TRN2", target_bir_lowering=False, debug=False,
                   num_devices=N_CORES)

    h = nc.dram_tensor("h", [R, H], f16, kind="ExternalInput")
    hfull = nc.dram_tensor("hfull", [S + 128, H], f32, kind="ExternalInput")
    wb = nc.dram_tensor("wb", [128, H], f16, kind="ExternalInput")
    wb32 = nc.dram_tensor("wb32", [128, H], f32, kind="ExternalInput")
    sel2 = nc.dram_tensor("sel2", [2, 256], f32, kind="ExternalInput")
    ones = nc.dram_tensor("ones", [128, 128], f32, kind="ExternalInput")
    ident = nc.dram_tensor("ident", [128, 128], f32, kind="ExternalInput")
    piv0 = nc.dram_tensor("piv0", [128, 1], f32, kind="ExternalInput")
    dcol = nc.dram_tensor("dcol", [128, N_LEVELS], f32, kind="ExternalInput")
    kconst = nc.dram_tensor("kconst", [128, 1], f32, kind="ExternalInput")
    ids1 = nc.dram_tensor("ids1", [16, 256], f32, kind="ExternalInput")
    kperm = nc.dram_tensor("kperm", [128, 1], f32, kind="ExternalInput")
    dumpg = nc.dram_tensor("dumpg", [128, 1], f32, kind="ExternalInput")
    rowid = nc.dram_tensor("rowid", [128, R], f32, kind="ExternalInput")
    rankoff = nc.dram_tensor("rankoff", [128, 1], f32, kind="ExternalInput")
    mcrow = nc.dram_tensor("mcrow", [128, 128 * N_REF], f32,
                           kind="ExternalInput")
    scores_out = nc.dram_tensor("scores_out", [RT, 128], f32,
                                kind="ExternalOutput")
    mask_out = nc.dram_tensor("mask_out", [RT, 128], f32,
                              kind="ExternalOutput")
    dbg = {}
    if _DBG:
        for nm, shape, dt in [
            ("d_tau", [128, 1], f32), ("d_acnt", [128, 1], f32),
            ("d_comp", [16, 8], f32), ("d_nf", [1, 1], u32),
            ("d_idxf", [128, 1], f32), ("d_valid", [128, 1], f32),
            ("d_se2", [128, 1], f32), ("d_theta", [128, 1], f32),
            ("d_bandbit", [128, 1], f32), ("d_scf", [128, 1], f32),
            ("d_band16", [16, 256], f32), ("d_sc16", [16, 256], f32),
            ("d_mdv", [16, 128], f32),
        ]:
            dbg[nm] = nc.dram_tensor(nm, shape, dt, kind="ExternalOutput")

    from contextlib import ExitStack

    with tile.TileContext(nc) as tc:
        with ExitStack() as stack:
            ec = stack.enter_context
            consts = ec(tc.tile_pool(name="consts", bufs=1))
            hp = ec(tc.tile_pool(name="hp",
                                 bufs=_HP_BUFS or max(2, 8 // NA)))
            junkvp = ec(tc.tile_pool(name="junkv", bufs=2))
            junkap = ec(tc.tile_pool(name="junka", bufs=1))
            jbp = ec(tc.tile_pool(name="jb", bufs=2))
            junk32p = ec(tc.tile_pool(name="junk32", bufs=1))
            onehotp = ec(tc.tile_pool(name="onehot", bufs=1))
            gathp = ec(tc.tile_pool(name="gath", bufs=2))
            scp = ec(tc.tile_pool(name="scp", bufs=2))
            flatp = ec(tc.tile_pool(name="flatp", bufs=3))
            agp = ec(tc.tile_pool(name="agp", bufs=3))
            bcp = ec(tc.tile_pool(name="bcp", bufs=2))
            smalls = ec(tc.tile_pool(name="smalls", bufs=2))
            refp = ec(tc.tile_pool(name="refp", bufs=2))
            pivp = ec(tc.tile_pool(name="pivp", bufs=8))
            pstp = ec(tc.tile_pool(name="pst", bufs=1, space="PSUM"))
            psbp = ec(tc.tile_pool(name="psb", bufs=2, space="PSUM"))
            psjp = ec(tc.tile_pool(name="psj", bufs=1, space="PSUM"))
            psTp = ec(tc.tile_pool(name="psT", bufs=1, space="PSUM"))
            psbbp = ec(tc.tile_pool(name="psbb", bufs=1, space="PSUM"))
            dram = ec(tc.tile_pool(name="dram", bufs=2, space="DRAM"))
            w_sb = consts.tile([128, H], f16)
            nc.sync.dma_start(w_sb[:], wb.ap())
            w32_sb = consts.tile([128, H], f32)
            nc.sync.dma_start(w32_sb[:], wb32.ap())
            sel2_sb = consts.tile([2, 256], f32)
            nc.sync.dma_start(sel2_sb[:], sel2.ap())
            ones_sb = consts.tile([128, 128], f32)
            nc.sync.dma_start(ones_sb[:], ones.ap())
            id_sb = consts.tile([128, 128], f32)
            nc.sync.dma_start(id_sb[:], ident.ap())
            piv0_sb = consts.tile([128, 1], f32)
            nc.sync.dma_start(piv0_sb[:], piv0.ap())
            dcol_sb = consts.tile([128, N_LEVELS], f32)
            nc.sync.dma_start(dcol_sb[:], dcol.ap())
            k_sb = consts.tile([128, 1], f32)
            nc.sync.dma_start(k_sb[:], kconst.ap())
            ids1_sb = consts.tile([16, 256], f32)
            nc.sync.dma_start(ids1_sb[:], ids1.ap())
            kperm_sb = consts.tile([128, 1], f32)
            nc.sync.dma_start(kperm_sb[:], kperm.ap())
            dumpg_sb = consts.tile([128, 1], f32)
            nc.sync.dma_start(dumpg_sb[:], dumpg.ap())
            rowid_sb = consts.tile([128, R], f32)
            nc.sync.dma_start(rowid_sb[:], rowid.ap())
            rankoff_sb = consts.tile([128, 1], f32)
            nc.sync.dma_start(rankoff_sb[:], rankoff.ap())
            mcrow_sb = consts.tile([128, 128 * N_REF], f32)
            nc.sync.dma_start(mcrow_sb[:], mcrow.ap())

            hv = h.ap().rearrange("(n a p) d -> n p a d", a=NA, p=128)

            def emit_matvec():
                """Phases 1-2: stream fp16 h, matvec on DVE, transpose."""
                scores_sb = scp.tile([128, RT], f32, tag="sc")
                flat_sc = flatp.tile([RT, 128], f32, tag="flat")
                rings = [nc.sync, nc.scalar][:_H_RINGS]
                n_chunks = RT // NA
                for i in range(n_chunks):
                    ht = hp.tile([128, NA, H], f16, tag="ht")
                    rings[i % len(rings)].dma_start(ht[:], hv[i])
                    for a in range(NA):
                        col = i * NA + a
                        junkv = junkvp.tile([128, H], f16, tag="jv")
                        nc.vector.scalar_tensor_tensor(
                            junkv[:, 0:_MATVEC_COLS],
                            ht[:, a, 0:_MATVEC_COLS], 0.0,
                            w_sb[:, 0:_MATVEC_COLS],
                            op0=Alu.bypass, op1=Alu.mult,
                            accum_out=scores_sb[:, col:col + 1],
                        )
                ps_t = pstp.tile([RT, 128], f32, tag="pst")
                nc.tensor.transpose(
                    ps_t[:], scores_sb[:, 0:RT], id_sb[:])
                nc.scalar.copy(flat_sc[0:RT, :], ps_t[:])
                return {"flat_sc": flat_sc}

            def emit_exchange(st):
                """Phase 3: scores out + pairwise AllGather (gpsimd ring)."""
                flat_sc = st["flat_sc"]
                nc.gpsimd.dma_start(scores_out.ap(), flat_sc[:])
                ag_in = dram.tile([RT, 128], f32, tag="agin")
                ag_out = dram.tile([2, R], f32, tag="agout")
                nc.gpsimd.dma_start(ag_in[:], flat_sc[:])
                nc.gpsimd.collective_compute(
                    "AllGather", mybir.AluOpType.bypass,
                    replica_groups=[[0, 1], [2, 3], [4, 5], [6, 7]],
                    ins=[ag_in.opt()], outs=[ag_out.opt()],
                )
                ag_sb = agp.tile([2, R], f32, tag="ag")
                nc.gpsimd.dma_start(ag_sb[:], ag_out[:])
                st["ag_sb"] = ag_sb
                st["ag_out"] = ag_out

            def emit_search(st):
                """Phases 4-6: broadcast, tau^ search, refinement, mask."""
                ag_sb = st["ag_sb"]
                ag_out = st["ag_out"]
                flat_sc = st["flat_sc"]
                bc_sb = bcp.tile([128, S], f32, tag="bc")
                for j in range(8):
                    ps_b = psbp.tile([128, 512], f32, tag="psb")
                    hh, nn = j // 4, j % 4
                    nc.tensor.matmul(
                        ps_b[:, :],
                        sel2_sb[:, hh * 128:(hh + 1) * 128],
                        ag_sb[:, nn * 512:(nn + 1) * 512],
                    )
                    nc.scalar.copy(
                        bc_sb[:, j * 512:(j + 1) * 512], ps_b[:])

                # start loading the flat batch-row scores for compaction
                sc16 = refp.tile([16, 256], f32, tag="sc16")
                nc.gpsimd.dma_start(
                    sc16[:], ag_out[:].rearrange("a (c f) -> (a c) f", f=256))

                piv = pivp.tile([128, 1], f32, tag="piv")
                nc.scalar.copy(piv[:], piv0_sb[:])
                for lvl in range(N_LEVELS):
                    junkv = junkvp.tile([128, H], f16, tag="jv")
                    cnt = smalls.tile([128, 1], f32, tag=f"cnt{lvl}")
                    nc.vector.tensor_scalar(
                        junkv[:, 0:S - N_ACT], bc_sb[:, 0:S - N_ACT],
                        piv[:, 0:1], None,
                        op0=Alu.is_ge, op1=Alu.add, accum_out=cnt[:],
                    )
                    junka = junkap.tile([128, N_ACT], f16, tag="ja")
                    sgn = smalls.tile([128, 1], f32, tag=f"sgn{lvl}")
                    nc.scalar.activation(
                        junka[:], bc_sb[:, S - N_ACT:S], Act.Sign,
                        bias=piv[:, 0:1], scale=-1.0, accum_out=sgn[:],
                    )
                    # cond = sign(2*cnt_dve - sgn - (2K - N_ACT) + 0.5)
                    t1 = smalls.tile([128, 1], f32, tag=f"t1{lvl}")
                    nc.vector.scalar_tensor_tensor(
                        t1[:], cnt[:], 2.0, sgn[:],
                        op0=Alu.mult, op1=Alu.subtract,
                    )
                    cond = smalls.tile([128, 1], f32, tag=f"cond{lvl}")
                    nc.scalar.activation(
                        cond[:], t1[:], Act.Sign, bias=k_sb[:, 0:1])
                    ps_j = psjp.tile([128, 1], f32, tag="psj")
                    nc.tensor.matmul(ps_j[:], ones_sb[:], cond[:])
                    e = smalls.tile([128, 1], f32, tag=f"e{lvl}")
                    nc.vector.tensor_tensor(
                        e[:], piv[:], dcol_sb[:, lvl:lvl + 1], op=Alu.add)
                    piv_n = pivp.tile([128, 1], f32, tag="piv")
                    nc.scalar.activation(
                        piv_n[:], ps_j[:, 0:1], Act.Identity,
                        bias=e[:, 0:1], scale=float(CS[lvl] / 2.0))
                    piv = piv_n
                # piv = tau^ replicated on all partitions

                tpl = smalls.tile([128, 1], f32, tag="tpl")
                nc.vector.tensor_scalar(
                    tpl[:], piv[:], float(DELTA), None, op0=Alu.add)
                tmn = smalls.tile([128, 1], f32, tag="tmn")
                nc.vector.tensor_scalar(
                    tmn[:], piv[:], float(-DELTA), None, op0=Alu.add)

                # exact count above the band: acnt = #(s^ > tau^ + DELTA).
                # Counted on the DMA-exact sc16 copy (bc_sb went through a
                # PE matmul and carries fp32r rounding); per-partition
                # counts are summed with a ones-matmul, exact on integers.
                jc16 = refp.tile([16, 256], f32, tag="jc16")
                acnt16 = refp.tile([16, 1], f32, tag="acnt16")
                nc.vector.tensor_scalar(
                    jc16[:], sc16[:], tpl[0:16, 0:1], None,
                    op0=Alu.is_gt, op1=Alu.add, accum_out=acnt16[:],
                )
                ps_ac = psjp.tile([128, 1], f32, tag="psj")
                nc.tensor.matmul(
                    ps_ac[:], ones_sb[0:16, :], acnt16[0:16, :])
                acnt = smalls.tile([128, 1], f32, tag="acnt")
                nc.scalar.copy(acnt[:], ps_ac[:])

                # band flags + candidate ids on the [16, 256] flat view
                bl = refp.tile([16, 256], f32, tag="bl")
                nc.vector.tensor_scalar(
                    bl[:], sc16[:], tpl[0:16, 0:1], None, op0=Alu.is_le)
                bg = refp.tile([16, 256], f32, tag="bg")
                nc.vector.tensor_scalar(
                    bg[:], sc16[:], tmn[0:16, 0:1], None, op0=Alu.is_ge)
                band16 = refp.tile([16, 256], f32, tag="band16")
                nc.vector.tensor_tensor(
                    band16[:], bl[:], bg[:], op=Alu.mult)
                v0 = refp.tile([16, 256], f32, tag="v0")
                nc.vector.scalar_tensor_tensor(
                    v0[:], band16[:], 0.0, ids1_sb[:],
                    op0=Alu.bypass, op1=Alu.mult)
                # val16 = band*(id+1) - 1 = id for band entries, -1 else
                val16 = refp.tile([16, 256], f32, tag="val16")
                nc.vector.tensor_scalar(
                    val16[:], v0[:], 1.0, None, op0=Alu.subtract)

                comp = refp.tile([16, 8], f32, tag="comp")
                nf = refp.tile([1, 1], u32, tag="nf")
                nc.gpsimd.sparse_gather(
                    comp[:], val16[:], num_found=nf[:])

                idxd = dram.tile([16, 8], f32, tag="idxd")
                nc.gpsimd.dma_start(idxd[:], comp[:])
                idxfr = refp.tile([128, 1], f32, tag="idxfr")
                nc.gpsimd.dma_start(
                    idxfr[:], idxd[:].rearrange("p (f o) -> (p f) o", o=1))

                nf32 = refp.tile([1, 1], f32, tag="nf32")
                nc.vector.tensor_copy(nf32[:], nf[:])
                nfb = refp.tile([128, 1], f32, tag="nfb")
                nc.gpsimd.partition_broadcast(nfb[:], nf32[:])

                valid = refp.tile([128, 1], f32, tag="valid")
                nc.vector.tensor_tensor(
                    valid[:], kperm_sb[:], nfb[:], op=Alu.is_lt)
                iv = refp.tile([128, 1], f32, tag="iv")
                nc.vector.tensor_scalar(
                    iv[:], valid[:], -1.0, 1.0, op0=Alu.mult, op1=Alu.add)
                # gather index: valid slot -> band id, else dump row
                i0 = refp.tile([128, 1], f32, tag="i0")
                nc.vector.tensor_tensor(
                    i0[:], idxfr[:], valid[:], op=Alu.mult)
                t1g = refp.tile([128, 1], f32, tag="t1g")
                nc.vector.tensor_tensor(
                    t1g[:], dumpg_sb[:], iv[:], op=Alu.mult)
                idxf = refp.tile([128, 1], f32, tag="idxf")
                nc.vector.tensor_tensor(
                    idxf[:], i0[:], t1g[:], op=Alu.add)
                idxi = refp.tile([128, 1], i32, tag="idxi")
                nc.vector.tensor_copy(idxi[:], idxf[:])

                gath = gathp.tile([128, H], f32, tag="gath")
                nc.gpsimd.indirect_dma_start(
                    out=gath[:], out_offset=None,
                    in_=hfull.ap(),
                    in_offset=IndirectOffsetOnAxis(ap=idxi[:, 0:1], axis=0),
                )
                junk32 = junk32p.tile([128, H], f32, tag="j32")
                se = refp.tile([128, 1], f32, tag="se")
                nc.vector.scalar_tensor_tensor(
                    junk32[:], gath[:], 0.0, w32_sb[:],
                    op0=Alu.bypass, op1=Alu.mult, accum_out=se[:])
                # invalid slots -> -BIG (se*valid is exact; (se+BIG)-BIG
                # would cancel to 0)
                se1 = refp.tile([128, 1], f32, tag="se1")
                nc.vector.tensor_tensor(
                    se1[:], se[:], valid[:], op=Alu.mult)
                sebig = refp.tile([128, 1], f32, tag="sebig")
                nc.vector.tensor_scalar(
                    sebig[:], iv[:], float(BIG), None, op0=Alu.mult)
                se2 = refp.tile([128, 1], f32, tag="se2")
                nc.vector.tensor_tensor(
                    se2[:], se1[:], sebig[:], op=Alu.subtract)

                # refinement threshold search with pivots along the FREE
                # axis: at each level the 128 candidate pivots are
                # base + m*c (mcrow const), so the count is a DVE compare
                # of (se2 - base) against mcrow plus one integer-exact PE
                # ones-matmul.  No partition broadcast of f32 values
                # anywhere (PE would round them through fp32r, and gpsimd
                # custom ops from a second Q7 library raced on HW).
                kneedb = refp.tile([128, 1], f32, tag="kneedb")
                nc.vector.tensor_scalar(
                    kneedb[:], acnt[:], float(0.5 - K_TOP), None,
                    op0=Alu.add)
                base = pivp.tile([128, 1], f32, tag="pivr")
                nc.vector.tensor_scalar(
                    base[:], piv[:], float(-DPRIME), None, op0=Alu.add)
                for lvl in range(N_REF):
                    serel = refp.tile([128, 1], f32, tag=f"serel{lvl}")
                    nc.vector.tensor_tensor(
                        serel[:], se2[:], base[:], op=Alu.subtract)
                    ind = refp.tile([128, 128], f32, tag=f"ind{lvl}")
                    nc.vector.tensor_tensor(
                        ind[:], serel[:].to_broadcast([128, 128]),
                        mcrow_sb[:, lvl * 128:(lvl + 1) * 128],
                        op=Alu.is_ge)
                    ps_c = psjp.tile([128, 128], f32, tag="psc")
                    nc.tensor.matmul(ps_c[:], ones_sb[:], ind[:])
                    cntrow = refp.tile([128, 128], f32, tag=f"cntrow{lvl}")
                    nc.scalar.copy(cntrow[:], ps_c[:])
                    condr = refp.tile([128, 128], f32, tag=f"condr{lvl}")
                    js = refp.tile([128, 1], f32, tag=f"js{lvl}")
                    nc.scalar.activation(
                        condr[:], cntrow[:], Act.Sign,
                        bias=kneedb[:, 0:1], accum_out=js[:])
                    # base' = base + (c/2)*js + 63c
                    er = refp.tile([128, 1], f32, tag=f"er{lvl}")
                    nc.vector.tensor_scalar(
                        er[:], base[:], float(63.0 * CSR[lvl]), None,
                        op0=Alu.add)
                    base_n = pivp.tile([128, 1], f32, tag="pivr")
                    nc.scalar.activation(
                        base_n[:], js[:], Act.Identity,
                        bias=er[:, 0:1], scale=float(CSR[lvl] / 2.0))
                    base = base_n
                # base = theta (lower edge of the final cell)

                # select with a half-final-cell downward nudge: the base
                # update and (se2 - base) compares carry a few ULP of f32
                # noise vs the direct compare below
                thsel = refp.tile([128, 1], f32, tag="thsel")
                nc.vector.tensor_scalar(
                    thsel[:], base[:], float(-CSR[N_REF - 1] / 2.0), None,
                    op0=Alu.add)
                bandbit = refp.tile([128, 1], f32, tag="bandbit")
                nc.vector.tensor_scalar(
                    bandbit[:], se2[:], thsel[:, 0:1], None, op0=Alu.is_ge)

                # scatter band bits to own-half positions with a one-hot
                # matmul (0/1 integers are exact in fp32r; no DRAM trip,
                # no HW queue races).  offf out of [0, R) matches no
                # column and contributes nothing.
                offf = refp.tile([128, 1], f32, tag="offf")
                nc.vector.tensor_tensor(
                    offf[:], idxf[:], rankoff_sb[:], op=Alu.subtract)
                onehot = onehotp.tile([128, R], f32, tag="onehot")
                nc.vector.tensor_tensor(
                    onehot[:], offf[:].to_broadcast([128, R]),
                    rowid_sb[:], op=Alu.is_equal)
                ps_ma = psjp.tile([128, RT], f32, tag="psma")
                for t in range(RT):
                    nc.tensor.matmul(
                        ps_ma[:, t:t + 1],
                        onehot[:, t * 128:(t + 1) * 128], bandbit[:, 0:1])
                ma_sb = scp.tile([128, RT], f32, tag="masb")
                nc.scalar.copy(ma_sb[:], ps_ma[:])
                ps_mt = pstp.tile([RT, 128], f32, tag="psmt")
                nc.tensor.transpose(ps_mt[:], ma_sb[:, 0:RT], id_sb[:])
                maT = flatp.tile([RT, 128], f32, tag="maT")
                nc.scalar.copy(maT[:], ps_mt[:])

                # merge: band-own entries hold their exact bit in maT and
                # have m0 = 0; everything else has maT = 0.
                m0 = flatp.tile([RT, 128], f32, tag="m0")
                nc.vector.tensor_scalar(
                    m0[:], flat_sc[:], tpl[0:RT, 0:1], None, op0=Alu.is_gt)
                mask_sb = flatp.tile([RT, 128], f32, tag="mask")
                nc.vector.tensor_tensor(
                    mask_sb[:], maT[:], m0[:], op=Alu.add)
                nc.gpsimd.dma_start(mask_out.ap(), mask_sb[:])
                if _DBG:
                    for nm, t in [
                        ("d_tau", piv), ("d_acnt", acnt), ("d_comp", comp),
                        ("d_nf", nf), ("d_idxf", idxf), ("d_valid", valid),
                        ("d_se2", se2), ("d_theta", base),
                        ("d_bandbit", bandbit), ("d_scf", offf),
                        ("d_band16", band16), ("d_sc16", sc16),
                        ("d_mdv", maT),
                    ]:
                        nc.gpsimd.dma_start(dbg[nm].ap(), t[:])

            # Software pipeline with a 1-rep skew (see f32 version).
            prev = None
            for rep in range(_REPS):
                st = emit_matvec()
                if prev is not None:
                    emit_search(prev)
                emit_exchange(st)
                prev = st
            emit_search(prev)

    nc.compile()
    return nc


def _host_inputs(hidden_states, gate_w):
    x = np.asarray(hidden_states, dtype=np.float32).reshape(B * S, H)
    w32 = np.asarray(gate_w, dtype=np.float32).reshape(1, H)
    flat16 = np.ascontiguousarray(x.astype(np.float16))
    wb16 = np.ascontiguousarray(
        np.broadcast_to(w32.astype(np.float16), (128, H)))
    wb32 = np.ascontiguousarray(np.broadcast_to(w32, (128, H)))
    sel2 = np.zeros((2, 256), np.float32)
    sel2[0, :128] = 1.0
    sel2[1, 128:] = 1.0
    ones = np.ones((128, 128), np.float32)
    ident = np.eye(128, dtype=np.float32)
    p = np.arange(128, dtype=np.float32)
    cs = [np.float32(c) for c in CS]
    piv0 = (np.float32(-W0 / 2.0) + p * cs[0]).reshape(128, 1)
    dcol = np.stack(
        [p * (cs[l + 1] - cs[l]) - cs[l] + 64.0 * cs[l]
         for l in range(N_LEVELS)],
        axis=1).astype(np.float32)
    kconst = np.full((128, 1), 0.5 - COMBINE_K, np.float32)
    # refinement constants
    ids1 = (np.arange(4096, dtype=np.float32) + 1.0).reshape(16, 256)
    q = np.arange(128)
    kperm = ((q % 8) * 16 + q // 8).astype(np.float32).reshape(128, 1)
    dumpg = (S + q).astype(np.float32).reshape(128, 1)
    rowid = np.broadcast_to(np.arange(R, dtype=np.float32), (128, R))
    rowid = np.ascontiguousarray(rowid)
    csr = [np.float32(c) for c in CSR]
    m = np.arange(128, dtype=np.float32)
    mcrow = np.concatenate(
        [np.broadcast_to(m * csr[l], (128, 128)) for l in range(N_REF)],
        axis=1).astype(np.float32)
    mcrow = np.ascontiguousarray(mcrow)

    in_maps = []
    for c in range(N_CORES):
        b = c // 2
        hfull = np.zeros((S + 128, H), np.float32)
        hfull[0:S] = x[b * S:(b + 1) * S]
        in_maps.append({
            "h": flat16[c * R:(c + 1) * R],
            "hfull": hfull,
            "wb": wb16,
            "wb32": wb32,
            "sel2": sel2,
            "ones": ones,
            "ident": ident,
            "piv0": piv0,
            "dcol": dcol,
            "kconst": kconst,
            "ids1": ids1,
            "kperm": kperm,
            "dumpg": dumpg,
            "rowid": rowid,
            "rankoff": np.full((128, 1), (c % 2) * R, np.float32),
            "mcrow": mcrow,
        })
    return in_maps


def _assemble(results):
    scores = np.concatenate(
        [results[c]["scores_out"].reshape(R) for c in range(N_CORES)]
    ).reshape(B, S)
    mask = np.concatenate(
        [results[c]["mask_out"].reshape(R) for c in range(N_CORES)]
    ).reshape(B, S)
    return mask, scores


def get_nc():
    if "nc" not in _CACHE:
        _CACHE["nc"] = _build_nc()
    return _CACHE["nc"]


def kernel(hidden_states, gate_w):
    from concourse.bass_utils import run_bass_kernel_spmd

    nc = get_nc()
    in_maps = _host_inputs(hidden_states, gate_w)
    res = run_bass_kernel_spmd(nc, in_maps, core_ids=list(range(N_CORES)))
    return _assemble(res.results)


# revision 29
# speedup vs baseline: 2.4126x; 2.4126x over previous
"""MoD router Trainium2 kernel (fp16-stream + exact-refinement version).

Computes, for hidden_states [4, 4096, 2048] and gate_w [1, 2048]:
    scores = einsum("bsh,h->bs", hidden_states, gate_w[0])        # [4, 4096]
    mask   = top-k mask per batch row (k = 2048 = S/2), 1.0/0.0   # [4, 4096]
returns (mask, scores), matching the reference.

The f32 kernel was pinned at the SBUF-fabric DMA ceiling (16.78 MB/core at
~436 GB/s = 38 us).  This version halves the stream: the host uploads the
per-core hidden slab as fp16 (8.39 MB), the DVE matvec runs on fp16
operands (f32 accumulate), and mask exactness vs the f32 reference is
restored with a refinement pass:

  1. Stream fp16 slab, DVE matvec -> approx scores s^ (err sigma ~7e-4);
     pairwise AllGather so each core holds its full 4096-score batch row.
  2. 2-level 127-ary search -> tau^ (within 1.3e-4 of the true median).
  3. Entries with |s^ - tau^| <= DELTA form the band (~40 of 4096; all
     possible misclassifications live there, DELTA >> max score error).
     Entries with s^ > tau^ + DELTA are certainly in the top-k.
  4. Band entry ids are compacted with gpsimd sparse_gather, their f32
     rows indirect-DMA-gathered from an f32 copy of the full batch row
     (uploaded host-side), rescored exactly on DVE.
  5. A 3-level 127-ary search over the <=128 exact band scores finds the
     (K - #above)-th largest (final cell 1.5e-8 < score spacing).  The
     level pivots live along the free axis as affine host constants and
     the cross-partition count is a ones-matmul on 0/1 indicators
     (integer-exact in fp32r); band bits return to their own-half flat
     positions via a one-hot is_equal matmul.  Everything stays in
     SBUF/PSUM: no partition_broadcast (second Q7 library -> reload race
     on HW) and no DRAM scatter round-trip.  Result: bit-exact top-k
     mask on the reference input.

Distribution: unchanged from the f32 version: B*S rows sharded 8 ways,
cores 2b/2b+1 own the halves of batch row b and exchange scores with a
pairwise AllGather (8 KB).  Both cores of a pair run the identical
deterministic search on identical data, so their decisions agree.

Timing probes: _MATVEC_COLS=1 strips DVE matvec work but keeps the DMA
stream; _REPS replicates the body inside one NEFF for delta timing.
"""

import numpy as np

B, S, H = 4, 4096, 2048
N_CORES = 8
R = (B * S) // N_CORES      # rows per core = 2048
RT = R // 128               # 128-row tiles per core = 16
K_TOP = S // 2              # 2048
NA = 2                      # row-tiles per DMA chunk (chunk = NA/2 MiB fp16)
N_LEVELS = 2                # tau^ search levels (127-ary, bracket [-1,1])
W0 = 2.0
CS = [W0 / (127.0 ** (l + 1)) for l in range(N_LEVELS)] + [0.0]
N_ACT = 3584                # cols counted on ACT in tau^ search (rest DVE)
COMBINE_K = float(2 * K_TOP - N_ACT)
DELTA = 0.012               # refinement band half-width (>> score err)
DPRIME = 1.25 * DELTA       # refinement search bracket half-width
N_REF = 3                   # refinement search levels
CSR = [2.0 * DPRIME / (127.0 ** (l + 1)) for l in range(N_REF)] + [0.0]
BIG = 1.0e30

_CACHE = {}
_DBG = False  # add debug DRAM outputs for refinement intermediates
_REPS = 1   # repeat whole body inside one NEFF (timing aid)
_MATVEC_COLS = H  # timing probe: set to 1 to keep DMA but skip DVE work
_H_RINGS = 2      # DMA rings for h chunks: sync+scalar HWDGE
_HP_BUFS = 0      # h-tile lookahead bufs (0 = default max(2, 10//NA))


def _build_nc():
    import concourse.bacc as bacc
    import concourse.tile as tile
    import concourse.mybir as mybir
    from concourse.bass import IndirectOffsetOnAxis

    f32 = mybir.dt.float32
    f16 = mybir.dt.float16
    i32 = mybir.dt.int32
    u32 = mybir.dt.uint32
    Alu = mybir.AluOpType
    Act = mybir.ActivationFunctionType

    nc = bacc.Bacc("TRN2", target_bir_lowering=False, debug=False,
                   num_devices=N_CORES)

    h = nc.dram_tensor("h", [R, H], f16, kind="ExternalInput")
    hfull = nc.dram_tensor("hfull", [S + 128, H], f32, kind="ExternalInput")
    wb = nc.dram_tensor("wb", [128, H], f16, kind="ExternalInput")
    wb32 = nc.dram_tensor("wb32", [128, H], f32, kind="ExternalInput")
    sel2 = nc.dram_tensor("sel2", [2, 256], f32, kind="ExternalInput")
    ones = nc.dram_tensor("ones", [128, 128], f32, kind="ExternalInput")
    ident = nc.dram_tensor("ident", [128, 128], f32, kind="ExternalInput")
    piv0 = nc.dram_tensor("piv0", [128, 1], f32, kind="ExternalInput")
    dcol = nc.dram_tensor("dcol", [128, N_LEVELS], f32, kind="ExternalInput")
    kconst = nc.dram_tensor("kconst", [128, 1], f32, kind="ExternalInput")
    ids1 = nc.dram_tensor("ids1", [16, 256], f32, kind="ExternalInput")
    kperm = nc.dram_tensor("kperm", [128, 1], f32, kind="ExternalInput")
    dumpg = nc.dram_tensor("dumpg", [128, 1], f32, kind="ExternalInput")
    rowid = nc.dram_tensor("rowid", [128, R], f32, kind="ExternalInput")
    rankoff = nc.dram_tensor("rankoff", [128, 1], f32, kind="ExternalInput")
    mcrow = nc.dram_tensor("mcrow", [128, 128 * N_REF], f32,
                           kind="ExternalInput")
    scores_out = nc.dram_tensor("scores_out", [RT, 128], f32,
                                kind="ExternalOutput")
    mask_out = nc.dram_tensor("mask_out", [RT, 128], f32,
                              kind="ExternalOutput")
    dbg = {}
    if _DBG:
        for nm, shape, dt in [
            ("d_tau", [128, 1], f32), ("d_acnt", [128, 1], f32),
            ("d_comp", [16, 8], f32), ("d_nf", [1, 1], u32),
            ("d_idxf", [128, 1], f32), ("d_valid", [128, 1], f32),
            ("d_se2", [128, 1], f32), ("d_theta", [128, 1], f32),
            ("d_bandbit", [128, 1], f32), ("d_scf", [128, 1], f32),
            ("d_band16", [16, 256], f32), ("d_sc16", [16, 256], f32),
            ("d_mdv", [16, 128], f32),
        ]:
            dbg[nm] = nc.dram_tensor(nm, shape, dt, kind="ExternalOutput")

    from contextlib import ExitStack

    with tile.TileContext(nc) as tc:
        with ExitStack() as stack:
            ec = stack.enter_context
            consts = ec(tc.tile_pool(name="consts", bufs=1))
            hp = ec(tc.tile_pool(name="hp",
                                 bufs=_HP_BUFS or max(2, 8 // NA)))
            junkvp = ec(tc.tile_pool(name="junkv", bufs=2))
            junkap = ec(tc.tile_pool(name="junka", bufs=1))
            jbp = ec(tc.tile_pool(name="jb", bufs=2))
            junk32p = ec(tc.tile_pool(name="junk32", bufs=1))
            onehotp = ec(tc.tile_pool(name="onehot", bufs=1))
            gathp = ec(tc.tile_pool(name="gath", bufs=2))
            scp = ec(tc.tile_pool(name="scp", bufs=2))
            flatp = ec(tc.tile_pool(name="flatp", bufs=3))
            agp = ec(tc.tile_pool(name="agp", bufs=3))
            bcp = ec(tc.tile_pool(name="bcp", bufs=2))
            smalls = ec(tc.tile_pool(name="smalls", bufs=2))
            refp = ec(tc.tile_pool(name="refp", bufs=2))
            pivp = ec(tc.tile_pool(name="pivp", bufs=8))
            pstp = ec(tc.tile_pool(name="pst", bufs=1, space="PSUM"))
            psbp = ec(tc.tile_pool(name="psb", bufs=2, space="PSUM"))
            psjp = ec(tc.tile_pool(name="psj", bufs=1, space="PSUM"))
            psTp = ec(tc.tile_pool(name="psT", bufs=1, space="PSUM"))
            psbbp = ec(tc.tile_pool(name="psbb", bufs=1, space="PSUM"))
            dram = ec(tc.tile_pool(name="dram", bufs=2, space="DRAM"))
            w_sb = consts.tile([128, H], f16)
            nc.sync.dma_start(w_sb[:], wb.ap())
            w32_sb = consts.tile([128, H], f32)
            nc.sync.dma_start(w32_sb[:], wb32.ap())
            sel2_sb = consts.tile([2, 256], f32)
            nc.sync.dma_start(sel2_sb[:], sel2.ap())
            ones_sb = consts.tile([128, 128], f32)
            nc.sync.dma_start(ones_sb[:], ones.ap())
            id_sb = consts.tile([128, 128], f32)
            nc.sync.dma_start(id_sb[:], ident.ap())
            piv0_sb = consts.tile([128, 1], f32)
            nc.sync.dma_start(piv0_sb[:], piv0.ap())
            dcol_sb = consts.tile([128, N_LEVELS], f32)
            nc.sync.dma_start(dcol_sb[:], dcol.ap())
            k_sb = consts.tile([128, 1], f32)
            nc.sync.dma_start(k_sb[:], kconst.ap())
            ids1_sb = consts.tile([16, 256], f32)
            nc.sync.dma_start(ids1_sb[:], ids1.ap())
            kperm_sb = consts.tile([128, 1], f32)
            nc.sync.dma_start(kperm_sb[:], kperm.ap())
            dumpg_sb = consts.tile([128, 1], f32)
            nc.sync.dma_start(dumpg_sb[:], dumpg.ap())
            rowid_sb = consts.tile([128, R], f32)
            nc.sync.dma_start(rowid_sb[:], rowid.ap())
            rankoff_sb = consts.tile([128, 1], f32)
            nc.sync.dma_start(rankoff_sb[:], rankoff.ap())
            mcrow_sb = consts.tile([128, 128 * N_REF], f32)
            nc.sync.dma_start(mcrow_sb[:], mcrow.ap())

            hv = h.ap().rearrange("(n a p) d -> n p a d", a=NA, p=128)

            def emit_matvec():
                """Phases 1-2: stream fp16 h, matvec on DVE, transpose."""
                scores_sb = scp.tile([128, RT], f32, tag="sc")
                flat_sc = flatp.tile([RT, 128], f32, tag="flat")
                rings = [nc.sync, nc.scalar][:_H_RINGS]
                n_chunks = RT // NA
                for i in range(n_chunks):
                    ht = hp.tile([128, NA, H], f16, tag="ht")
                    rings[i % len(rings)].dma_start(ht[:], hv[i])
                    for a in range(NA):
                        col = i * NA + a
                        junkv = junkvp.tile([128, H], f16, tag="jv")
                        nc.vector.scalar_tensor_tensor(
                            junkv[:, 0:_MATVEC_COLS],
                            ht[:, a, 0:_MATVEC_COLS], 0.0,
                            w_sb[:, 0:_MATVEC_COLS],
                            op0=Alu.bypass, op1=Alu.mult,
                            accum_out=scores_sb[:, col:col + 1],
                        )
                ps_t = pstp.tile([RT, 128], f32, tag="pst")
                nc.tensor.transpose(
                    ps_t[:], scores_sb[:, 0:RT], id_sb[:])
                nc.scalar.copy(flat_sc[0:RT, :], ps_t[:])
                return {"flat_sc": flat_sc}

            def emit_exchange(st):
                """Phase 3: scores out + pairwise AllGather (gpsimd ring)."""
                flat_sc = st["flat_sc"]
                nc.gpsimd.dma_start(scores_out.ap(), flat_sc[:])
                ag_in = dram.tile([RT, 128], f32, tag="agin")
                ag_out = dram.tile([2, R], f32, tag="agout")
                nc.gpsimd.dma_start(ag_in[:], flat_sc[:])
                nc.gpsimd.collective_compute(
                    "AllGather", mybir.AluOpType.bypass,
                    replica_groups=[[0, 1], [2, 3], [4, 5], [6, 7]],
                    ins=[ag_in.opt()], outs=[ag_out.opt()],
                )
                ag_sb = agp.tile([2, R], f32, tag="ag")
                nc.gpsimd.dma_start(ag_sb[:], ag_out[:])
                st["ag_sb"] = ag_sb
                st["ag_out"] = ag_out

            def emit_search(st):
                """Phases 4-6: broadcast, tau^ search, refinement, mask."""
                ag_sb = st["ag_sb"]
                ag_out = st["ag_out"]
                flat_sc = st["flat_sc"]
                bc_sb = bcp.tile([128, S], f32, tag="bc")
                for j in range(8):
                    ps_b = psbp.tile([128, 512], f32, tag="psb")
                    hh, nn = j // 4, j % 4
                    nc.tensor.matmul(
                        ps_b[:, :],
                        sel2_sb[:, hh * 128:(hh + 1) * 128],
                        ag_sb[:, nn * 512:(nn + 1) * 512],
                    )
                    nc.scalar.copy(
                        bc_sb[:, j * 512:(j + 1) * 512], ps_b[:])

                # start loading the flat batch-row scores for compaction
                sc16 = refp.tile([16, 256], f32, tag="sc16")
                nc.gpsimd.dma_start(
                    sc16[:], ag_out[:].rearrange("a (c f) -> (a c) f", f=256))

                piv = pivp.tile([128, 1], f32, tag="piv")
                nc.scalar.copy(piv[:], piv0_sb[:])
                for lvl in range(N_LEVELS):
                    junkv = junkvp.tile([128, H], f16, tag="jv")
                    cnt = smalls.tile([128, 1], f32, tag=f"cnt{lvl}")
                    nc.vector.tensor_scalar(
                        junkv[:, 0:S - N_ACT], bc_sb[:, 0:S - N_ACT],
                        piv[:, 0:1], None,
                        op0=Alu.is_ge, op1=Alu.add, accum_out=cnt[:],
                    )
                    junka = junkap.tile([128, N_ACT], f16, tag="ja")
                    sgn = smalls.tile([128, 1], f32, tag=f"sgn{lvl}")
                    nc.scalar.activation(
                        junka[:], bc_sb[:, S - N_ACT:S], Act.Sign,
                        bias=piv[:, 0:1], scale=-1.0, accum_out=sgn[:],
                    )
                    # cond = sign(2*cnt_dve - sgn - (2K - N_ACT) + 0.5)
                    t1 = smalls.tile([128, 1], f32, tag=f"t1{lvl}")
                    nc.vector.scalar_tensor_tensor(
                        t1[:], cnt[:], 2.0, sgn[:],
                        op0=Alu.mult, op1=Alu.subtract,
                    )
                    cond = smalls.tile([128, 1], f32, tag=f"cond{lvl}")
                    nc.scalar.activation(
                        cond[:], t1[:], Act.Sign, bias=k_sb[:, 0:1])
                    ps_j = psjp.tile([128, 1], f32, tag="psj")
                    nc.tensor.matmul(ps_j[:], ones_sb[:], cond[:])
                    e = smalls.tile([128, 1], f32, tag=f"e{lvl}")
                    nc.vector.tensor_tensor(
                        e[:], piv[:], dcol_sb[:, lvl:lvl + 1], op=Alu.add)
                    piv_n = pivp.tile([128, 1], f32, tag="piv")
                    nc.scalar.activation(
                        piv_n[:], ps_j[:, 0:1], Act.Identity,
                        bias=e[:, 0:1], scale=float(CS[lvl] / 2.0))
                    piv = piv_n
                # piv = tau^ replicated on all partitions

                tpl = smalls.tile([128, 1], f32, tag="tpl")
                nc.vector.tensor_scalar(
                    tpl[:], piv[:], float(DELTA), None, op0=Alu.add)
                tmn = smalls.tile([128, 1], f32, tag="tmn")
                nc.vector.tensor_scalar(
                    tmn[:], piv[:], float(-DELTA), None, op0=Alu.add)

                # exact count above the band: acnt = #(s^ > tau^ + DELTA).
                # Counted on the DMA-exact sc16 copy (bc_sb went through a
                # PE matmul and carries fp32r rounding); per-partition
                # counts are summed with a ones-matmul, exact on integers.
                jc16 = refp.tile([16, 256], f32, tag="jc16")
                acnt16 = refp.tile([16, 1], f32, tag="acnt16")
                nc.vector.tensor_scalar(
                    jc16[:], sc16[:], tpl[0:16, 0:1], None,
                    op0=Alu.is_gt, op1=Alu.add, accum_out=acnt16[:],
                )
                ps_ac = psjp.tile([128, 1], f32, tag="psj")
                nc.tensor.matmul(
                    ps_ac[:], ones_sb[0:16, :], acnt16[0:16, :])
                acnt = smalls.tile([128, 1], f32, tag="acnt")
                nc.scalar.copy(acnt[:], ps_ac[:])

                # band flags + candidate ids on the [16, 256] flat view
                bl = refp.tile([16, 256], f32, tag="bl")
                nc.vector.tensor_scalar(
                    bl[:], sc16[:], tpl[0:16, 0:1], None, op0=Alu.is_le)
                bg = refp.tile([16, 256], f32, tag="bg")
                nc.vector.tensor_scalar(
                    bg[:], sc16[:], tmn[0:16, 0:1], None, op0=Alu.is_ge)
                band16 = refp.tile([16, 256], f32, tag="band16")
                nc.vector.tensor_tensor(
                    band16[:], bl[:], bg[:], op=Alu.mult)
                v0 = refp.tile([16, 256], f32, tag="v0")
                nc.vector.scalar_tensor_tensor(
                    v0[:], band16[:], 0.0, ids1_sb[:],
                    op0=Alu.bypass, op1=Alu.mult)
                # val16 = band*(id+1) - 1 = id for band entries, -1 else
                val16 = refp.tile([16, 256], f32, tag="val16")
                nc.vector.tensor_scalar(
                    val16[:], v0[:], 1.0, None, op0=Alu.subtract)

                comp = refp.tile([16, 8], f32, tag="comp")
                nf = refp.tile([1, 1], u32, tag="nf")
                nc.gpsimd.sparse_gather(
                    comp[:], val16[:], num_found=nf[:])

                idxd = dram.tile([16, 8], f32, tag="idxd")
                nc.gpsimd.dma_start(idxd[:], comp[:])
                idxfr = refp.tile([128, 1], f32, tag="idxfr")
                nc.gpsimd.dma_start(
                    idxfr[:], idxd[:].rearrange("p (f o) -> (p f) o", o=1))

                nf32 = refp.tile([1, 1], f32, tag="nf32")
                nc.vector.tensor_copy(nf32[:], nf[:])
                # replicate the (integer) count across partitions with a
                # ones-matmul: fp32r-exact, and avoids partition_broadcast
                # (a second Q7 library whose per-rep reload is expensive
                # and raced on HW)
                ps_nf = psjp.tile([128, 1], f32, tag="psnf")
                nc.tensor.matmul(
                    ps_nf[:], ones_sb[0:1, :], nf32[0:1, 0:1])
                nfb = refp.tile([128, 1], f32, tag="nfb")
                nc.scalar.copy(nfb[:], ps_nf[:])

                valid = refp.tile([128, 1], f32, tag="valid")
                nc.vector.tensor_tensor(
                    valid[:], kperm_sb[:], nfb[:], op=Alu.is_lt)
                iv = refp.tile([128, 1], f32, tag="iv")
                nc.vector.tensor_scalar(
                    iv[:], valid[:], -1.0, 1.0, op0=Alu.mult, op1=Alu.add)
                # gather index: valid slot -> band id, else dump row
                i0 = refp.tile([128, 1], f32, tag="i0")
                nc.vector.tensor_tensor(
                    i0[:], idxfr[:], valid[:], op=Alu.mult)
                t1g = refp.tile([128, 1], f32, tag="t1g")
                nc.vector.tensor_tensor(
                    t1g[:], dumpg_sb[:], iv[:], op=Alu.mult)
                idxf = refp.tile([128, 1], f32, tag="idxf")
                nc.vector.tensor_tensor(
                    idxf[:], i0[:], t1g[:], op=Alu.add)
                idxi = refp.tile([128, 1], i32, tag="idxi")
                nc.vector.tensor_copy(idxi[:], idxf[:])

                gath = gathp.tile([128, H], f32, tag="gath")
                nc.gpsimd.indirect_dma_start(
                    out=gath[:], out_offset=None,
                    in_=hfull.ap(),
                    in_offset=IndirectOffsetOnAxis(ap=idxi[:, 0:1], axis=0),
                )
                junk32 = junk32p.tile([128, H], f32, tag="j32")
                se = refp.tile([128, 1], f32, tag="se")
                nc.vector.scalar_tensor_tensor(
                    junk32[:], gath[:], 0.0, w32_sb[:],
                    op0=Alu.bypass, op1=Alu.mult, accum_out=se[:])
                # invalid slots -> -BIG (se*valid is exact; (se+BIG)-BIG
                # would cancel to 0)
                se1 = refp.tile([128, 1], f32, tag="se1")
                nc.vector.tensor_tensor(
                    se1[:], se[:], valid[:], op=Alu.mult)
                sebig = refp.tile([128, 1], f32, tag="sebig")
                nc.vector.tensor_scalar(
                    sebig[:], iv[:], float(BIG), None, op0=Alu.mult)
                se2 = refp.tile([128, 1], f32, tag="se2")
                nc.vector.tensor_tensor(
                    se2[:], se1[:], sebig[:], op=Alu.subtract)

                # refinement threshold search with pivots along the FREE
                # axis: at each level the 128 candidate pivots are
                # base + m*c (mcrow const), so the count is a DVE compare
                # of (se2 - base) against mcrow plus one integer-exact PE
                # ones-matmul.  No partition broadcast of f32 values
                # anywhere (PE would round them through fp32r, and gpsimd
                # custom ops from a second Q7 library raced on HW).
                kneedb = refp.tile([128, 1], f32, tag="kneedb")
                nc.vector.tensor_scalar(
                    kneedb[:], acnt[:], float(0.5 - K_TOP), None,
                    op0=Alu.add)
                base = pivp.tile([128, 1], f32, tag="pivr")
                nc.vector.tensor_scalar(
                    base[:], piv[:], float(-DPRIME), None, op0=Alu.add)
                for lvl in range(N_REF):
                    serel = refp.tile([128, 1], f32, tag=f"serel{lvl}")
                    nc.vector.tensor_tensor(
                        serel[:], se2[:], base[:], op=Alu.subtract)
                    ind = refp.tile([128, 128], f32, tag=f"ind{lvl}")
                    nc.vector.tensor_tensor(
                        ind[:], serel[:].to_broadcast([128, 128]),
                        mcrow_sb[:, lvl * 128:(lvl + 1) * 128],
                        op=Alu.is_ge)
                    ps_c = psjp.tile([128, 128], f32, tag="psc")
                    nc.tensor.matmul(ps_c[:], ones_sb[:], ind[:])
                    cntrow = refp.tile([128, 128], f32, tag=f"cntrow{lvl}")
                    nc.scalar.copy(cntrow[:], ps_c[:])
                    condr = refp.tile([128, 128], f32, tag=f"condr{lvl}")
                    js = refp.tile([128, 1], f32, tag=f"js{lvl}")
                    nc.scalar.activation(
                        condr[:], cntrow[:], Act.Sign,
                        bias=kneedb[:, 0:1], accum_out=js[:])
                    # base' = base + (c/2)*js + 63c
                    er = refp.tile([128, 1], f32, tag=f"er{lvl}")
                    nc.vector.tensor_scalar(
                        er[:], base[:], float(63.0 * CSR[lvl]), None,
                        op0=Alu.add)
                    base_n = pivp.tile([128, 1], f32, tag="pivr")
                    nc.scalar.activation(
                        base_n[:], js[:], Act.Identity,
                        bias=er[:, 0:1], scale=float(CSR[lvl] / 2.0))
                    base = base_n
                # base = theta (lower edge of the final cell)

                # select with a half-final-cell downward nudge: the base
                # update and (se2 - base) compares carry a few ULP of f32
                # noise vs the direct compare below
                thsel = refp.tile([128, 1], f32, tag="thsel")
                nc.vector.tensor_scalar(
                    thsel[:], base[:], float(-CSR[N_REF - 1] / 2.0), None,
                    op0=Alu.add)
                bandbit = refp.tile([128, 1], f32, tag="bandbit")
                nc.vector.tensor_scalar(
                    bandbit[:], se2[:], thsel[:, 0:1], None, op0=Alu.is_ge)

                # scatter band bits to own-half positions with a one-hot
                # matmul (0/1 integers are exact in fp32r; no DRAM trip,
                # no HW queue races).  offf out of [0, R) matches no
                # column and contributes nothing.
                offf = refp.tile([128, 1], f32, tag="offf")
                nc.vector.tensor_tensor(
                    offf[:], idxf[:], rankoff_sb[:], op=Alu.subtract)
                onehot = onehotp.tile([128, R], f32, tag="onehot")
                nc.vector.tensor_tensor(
                    onehot[:], offf[:].to_broadcast([128, R]),
                    rowid_sb[:], op=Alu.is_equal)
                ps_ma = psjp.tile([128, RT], f32, tag="psma")
                for t in range(RT):
                    nc.tensor.matmul(
                        ps_ma[:, t:t + 1],
                        onehot[:, t * 128:(t + 1) * 128], bandbit[:, 0:1])
                ma_sb = scp.tile([128, RT], f32, tag="masb")
                nc.scalar.copy(ma_sb[:], ps_ma[:])
                ps_mt = pstp.tile([RT, 128], f32, tag="psmt")
                nc.tensor.transpose(ps_mt[:], ma_sb[:, 0:RT], id_sb[:])
                maT = flatp.tile([RT, 128], f32, tag="maT")
                nc.scalar.copy(maT[:], ps_mt[:])

                # merge: band-own entries hold their exact bit in maT and
                # have m0 = 0; everything else has maT = 0.
                m0 = flatp.tile([RT, 128], f32, tag="m0")
                nc.vector.tensor_scalar(
                    m0[:], flat_sc[:], tpl[0:RT, 0:1], None, op0=Alu.is_gt)
                mask_sb = flatp.tile([RT, 128], f32, tag="mask")
                nc.vector.tensor_tensor(
                    mask_sb[:], maT[:], m0[:], op=Alu.add)
                nc.gpsimd.dma_start(mask_out.ap(), mask_sb[:])
                if _DBG:
                    for nm, t in [
                        ("d_tau", piv), ("d_acnt", acnt), ("d_comp", comp),
                        ("d_nf", nf), ("d_idxf", idxf), ("d_valid", valid),
                        ("d_se2", se2), ("d_theta", base),
                        ("d_bandbit", bandbit), ("d_scf", offf),
                        ("d_band16", band16), ("d_sc16", sc16),
                        ("d_mdv", maT),
                    ]:
                        nc.gpsimd.dma_start(dbg[nm].ap(), t[:])

            # Software pipeline with a 1-rep skew (see f32 version).
            prev = None
            for rep in range(_REPS):
                st = emit_matvec()
                if prev is not None:
                    emit_search(prev)
                emit_exchange(st)
                prev = st
            emit_search(prev)

    nc.compile()
    return nc


def _host_inputs(hidden_states, gate_w):
    x = np.asarray(hidden_states, dtype=np.float32).reshape(B * S, H)
    w32 = np.asarray(gate_w, dtype=np.float32).reshape(1, H)
    flat16 = np.ascontiguousarray(x.astype(np.float16))
    wb16 = np.ascontiguousarray(
        np.broadcast_to(w32.astype(np.float16), (128, H)))
    wb32 = np.ascontiguousarray(np.broadcast_to(w32, (128, H)))
    sel2 = np.zeros((2, 256), np.float32)
    sel2[0, :128] = 1.0
    sel2[1, 128:] = 1.0
    ones = np.ones((128, 128), np.float32)
    ident = np.eye(128, dtype=np.float32)
    p = np.arange(128, dtype=np.float32)
    cs = [np.float32(c) for c in CS]
    piv0 = (np.float32(-W0 / 2.0) + p * cs[0]).reshape(128, 1)
    dcol = np.stack(
        [p * (cs[l + 1] - cs[l]) - cs[l] + 64.0 * cs[l]
         for l in range(N_LEVELS)],
        axis=1).astype(np.float32)
    kconst = np.full((128, 1), 0.5 - COMBINE_K, np.float32)
    # refinement constants
    ids1 = (np.arange(4096, dtype=np.float32) + 1.0).reshape(16, 256)
    q = np.arange(128)
    kperm = ((q % 8) * 16 + q // 8).astype(np.float32).reshape(128, 1)
    dumpg = (S + q).astype(np.float32).reshape(128, 1)
    rowid = np.broadcast_to(np.arange(R, dtype=np.float32), (128, R))
    rowid = np.ascontiguousarray(rowid)
    csr = [np.float32(c) for c in CSR]
    m = np.arange(128, dtype=np.float32)
    mcrow = np.concatenate(
        [np.broadcast_to(m * csr[l], (128, 128)) for l in range(N_REF)],
        axis=1).astype(np.float32)
    mcrow = np.ascontiguousarray(mcrow)

    in_maps = []
    for c in range(N_CORES):
        b = c // 2
        hfull = np.zeros((S + 128, H), np.float32)
        hfull[0:S] = x[b * S:(b + 1) * S]
        in_maps.append({
            "h": flat16[c * R:(c + 1) * R],
            "hfull": hfull,
            "wb": wb16,
            "wb32": wb32,
            "sel2": sel2,
            "ones": ones,
            "ident": ident,
            "piv0": piv0,
            "dcol": dcol,
            "kconst": kconst,
            "ids1": ids1,
            "kperm": kperm,
            "dumpg": dumpg,
            "rowid": rowid,
            "rankoff": np.full((128, 1), (c % 2) * R, np.float32),
            "mcrow": mcrow,
        })
    return in_maps


def _assemble(results):
    scores = np.concatenate(
        [results[c]["scores_out"].reshape(R) for c in range(N_CORES)]
    ).reshape(B, S)
    mask = np.concatenate(
        [results[c]["mask_out"].reshape(R) for c in range(N_CORES)]
    ).reshape(B, S)
    return mask, scores


def get_nc():
    if "nc" not in _CACHE:
        _CACHE["nc"] = _build_nc()
    return _CACHE["nc"]


def kernel(hidden_states, gate_w):
    from concourse.bass_utils import run_bass_kernel_spmd

    nc = get_nc()
    in_maps = _host_inputs(hidden_states, gate_w)
    res = run_bass_kernel_spmd(nc, in_maps, core_ids=list(range(N_CORES)))
    return _assemble(res.results)
